# revision 10
# baseline (speedup 1.0000x reference)
"""Additive (Bahdanau) attention on 8 Trainium2 NeuronCores.

Reference math (per batch b):
    qh = queries @ Wq                  (NQ, H)
    kh = keys    @ Wk                  (NK, H)
    scores[q,k] = sum_h wv[h] * tanh(qh[q,h] + kh[k,h])
    attn = softmax(mask(scores))       mask: k >= valid_len -> -1e6
    out  = attn @ values               (NQ, V)

Sharding: 8 cores = 4 batches x 2 query-halves (128 q-rows each). Each core
owns the full key dimension -> no collectives, host just concatenates.

Per-core device algorithm (NQS=128 q, NK=2048 k, H=32):
  - partitions carry (j, h) = (q mod 4, h)  -> 4*32 = 128 lanes
  - kh4 psum (128, 2048): kh replicated 4x over partition groups, computed by
    4 col-tiled matmuls lhsT=Wk rhs=keys^T
  - qh4 sbuf (128, 32): qh4[(j,h), g] = qh[4g+j, h] via 4 col-tiled matmuls
  - per q-group g (32 groups of 4 q's):
      F_g = tanh(kh4 + bias qh4[:, g])        one ScalarE pass, FD=2048
      scores[4g:4g+4, :] += wv-weighted h-reduction: TensorE matmul with a
      zero-padded (128,128) stationary weight accumulating into scores psum
  - P = exp(scores)  (no max-subtraction needed: |scores| <= ||wv||_1 ~ 5)
  - transpose P via PE, multiply by 0/1 mask column (per-partition scalar)
  - out_unnorm (128, 65) = P_T.T @ [V | 1] accumulated over 16 k-tiles;
    column 64 is the masked softmax denominator l
  - out = out_unnorm[:, :64] * (1/l)

Masked keys contribute exactly 0 (mask multiply) and the missing max
subtraction cancels in the p/l ratio, so this matches the reference exactly
up to fp32 rounding.
"""

import ml_dtypes
import numpy as np

import concourse.bacc as bacc
import concourse.tile as tile
from concourse import mybir
from concourse.bass_utils import run_bass_kernel_spmd

B, NQ, NK = 4, 256, 2048
QKD, H, VD = 64, 32, 64
NQS = 128          # q rows per core
NG = NQS // 4      # 32 q-groups of 4
NKT = NK // 128    # 16 k-tiles
F32 = mybir.dt.float32
BF16 = mybir.dt.bfloat16

_cache = {}


def _build_nc():
    nc = bacc.Bacc("TRN2", debug=False, num_devices=8)

    # blob columns: [0:32]=wq, [32:64]=wk, [64:192]=qTr, [192:320]=ident,
    # [320:336]=maskc  (wq/wk/qTr occupy partitions 0-63)
    d_kT = nc.declare_dram_parameter("kT", [128, NK // 2], BF16, isOutput=False)
    d_blob = nc.declare_dram_parameter("blob", [128, 336], BF16, isOutput=False)
    d_wvb = nc.declare_dram_parameter("wvb", [128, NG * 32], BF16, isOutput=False)
    d_vaug = nc.declare_dram_parameter("vaug", [128, NKT * 65], BF16, isOutput=False)
    d_out = nc.declare_dram_parameter("out", [NQS, VD], F32, isOutput=True)

    TANH = mybir.ActivationFunctionType.Tanh
    EXP = mybir.ActivationFunctionType.Exp

    with tile.TileContext(nc) as tc:
        with (
            tc.tile_pool(name="sb", bufs=1) as sb,
            tc.tile_pool(name="fpool", bufs=2) as fpool,
            tc.tile_pool(name="psA", bufs=1, space="PSUM") as psA,
            tc.tile_pool(name="psB", bufs=1, space="PSUM") as psB,
        ):
            # ---- constant / input tiles ----
            # kT2: [0:64, f] = keys^T[:, f], [64:128, f] = keys^T[:, 1024+f]
            kT_sb = sb.tile([128, NK // 2], BF16, tag="kT")
            blob_sb = sb.tile([128, 336], BF16, tag="blob")
            wvb_sb = sb.tile([128, NG * 32], BF16, tag="wvb")
            vaug_sb = sb.tile([128, NKT * 65], BF16, tag="vaug")
            qh4_sb = sb.tile([128, NG], F32, tag="qh4")
            kh4bf_sb = sb.tile([128, NK], BF16, tag="kh4bf")
            wq_sb = blob_sb[0:QKD, 0:32]
            wk_lo = blob_sb[0:QKD, 32:64]
            wk_hi = blob_sb[QKD:128, 32:64]
            qTr_sb = blob_sb[0:QKD, 64:192]
            ident_sb = blob_sb[:, 192:320]
            maskc_bf = blob_sb[:, 320:336]
            maskc_sb = sb.tile([128, NKT], F32, tag="maskf")
            P_sb = sb.tile([128, NK], BF16, tag="P")
            PT_sb = sb.tile([128, NK], BF16, tag="PT")
            linv_sb = sb.tile([128, 1], F32, tag="linv")
            out_sb = sb.tile([NQS, VD], F32, tag="outsb")

            for c in range(2):
                nc.sync.dma_start(
                    out=kT_sb[:, c * 512:(c + 1) * 512],
                    in_=d_kT[:, c * 512:(c + 1) * 512],
                )
            nc.gpsimd.dma_start(out=blob_sb[:], in_=d_blob[:])
            nc.gpsimd.dma_start(out=wvb_sb[:], in_=d_wvb[:])
            nc.gpsimd.dma_start(out=vaug_sb[:], in_=d_vaug[:])
            nc.vector.tensor_copy(maskc_sb[:], maskc_bf)

            # ---- psum tiles ----
            kh4_ps = psA.tile([128, NK], F32, tag="big")
            qh4_ps = psB.tile([128, NG], F32, tag="acc")

            # qh4[(j,h), g] = sum_d Wq[d,h] * qTr[d, j*32+g]
            for j in range(4):
                nc.tensor.matmul(
                    qh4_ps[32 * j:32 * (j + 1), :],
                    lhsT=wq_sb,
                    rhs=qTr_sb[:, j * 32:(j + 1) * 32],
                    start=True, stop=True,
                    tile_position=(0, 32 * j),
                )
            nc.vector.tensor_copy(qh4_sb[:], qh4_ps[:])

            # kh4[(j,h), k] = sum_d Wk[d,h] * kT[d,k]  (replicated over j),
            # then narrowed to bf16 in SBUF per 512-chunk
            for c in range(4):
                src_rows = 0 if c < 2 else QKD
                rhs = kT_sb[src_rows:src_rows + QKD, (c % 2) * 512:(c % 2 + 1) * 512]
                wk = wk_lo if c < 2 else wk_hi
                for j in range(4):
                    nc.tensor.matmul(
                        kh4_ps[32 * j:32 * (j + 1), c * 512:(c + 1) * 512],
                        lhsT=wk,
                        rhs=rhs,
                        start=True, stop=True,
                        tile_position=(src_rows, 32 * j),
                    )
                nc.vector.tensor_copy(
                    kh4bf_sb[:, c * 512:(c + 1) * 512],
                    kh4_ps[:, c * 512:(c + 1) * 512],
                )

            # ---- main loop: DVE bias-add -> one big in-place tanh per chunk
            # -> TensorE h-reduction. Ramped chunk sizes keep startup short.
            scores_ps = psB.tile([128, NK], F32, tag="acc")
            CHUNKS = [2, 2, 4, 8, 8, 8]
            g = 0
            for nch in CHUNKS:
                Fs = fpool.tile([128, nch * NK], BF16, tag=f"Fs{nch}",
                                bufs=2, name=f"Fs_{g}")
                for i in range(nch):
                    nc.vector.tensor_scalar_add(
                        Fs[:, i * NK:(i + 1) * NK], kh4bf_sb[:],
                        qh4_sb[:, g + i:g + i + 1],
                    )
                nc.scalar.activation(Fs[:], Fs[:], TANH)
                for i in range(nch):
                    gg = g + i
                    G = gg // 8
                    for c in range(4):
                        nc.tensor.matmul(
                            scores_ps[32 * G:32 * (G + 1), c * 512:(c + 1) * 512],
                            lhsT=wvb_sb[:, gg * 32:(gg + 1) * 32],
                            rhs=Fs[:, i * NK + c * 512:i * NK + (c + 1) * 512],
                            start=(gg % 8 == 0), stop=(gg % 8 == 7),
                            skip_group_check=True,
                            tile_position=(0, 32 * G),
                        )
                g += nch

            # ---- softmax numerator ----
            nc.scalar.activation(P_sb[:], scores_ps[:], EXP)

            # ---- transpose P (PE) + mask multiply (DVE) + AV matmul ----
            PT_ps = psA.tile([128, 2 * NK], BF16, tag="big")
            av_ps = psB.tile([128, 65], F32, tag="acc")
            for t in range(NKT):
                off = (t % 4) * 1024 + (t // 4) * 128
                nc.tensor.transpose(
                    PT_ps[:, off:off + 128],
                    P_sb[:, t * 128:(t + 1) * 128],
                    ident_sb,
                )
                nc.vector.tensor_scalar_mul(
                    PT_sb[:, t * 128:(t + 1) * 128],
                    PT_ps[:, off:off + 128],
                    maskc_sb[:, t:t + 1],
                )
                nc.tensor.matmul(
                    av_ps[:],
                    lhsT=PT_sb[:, t * 128:(t + 1) * 128],
                    rhs=vaug_sb[:, t * 65:(t + 1) * 65],
                    start=(t == 0), stop=(t == NKT - 1),
                )

            # ---- normalize + store ----
            nc.vector.reciprocal(linv_sb[:], av_ps[:, 64:65])
            nc.vector.tensor_scalar_mul(out_sb[:], av_ps[:, 0:64], linv_sb[:])
            nc.sync.dma_start(out=d_out[:], in_=out_sb[:])

    nc.compile()
    return nc


def _host_shards(queries, keys, values, valid_lens, Wq, Wk, wv):
    """Pure data-marshaling: shard, transpose layouts, build mask/weight
    layouts. All FLOPs on the actual tensors happen on device."""
    f32 = np.float32
    queries = np.asarray(queries, f32)
    keys = np.asarray(keys, f32)
    values = np.asarray(values, f32)
    valid_lens = np.asarray(valid_lens)
    Wq = np.asarray(Wq, f32)
    Wk = np.asarray(Wk, f32)
    wv = np.asarray(wv, f32)

    # zero-padded stationary weights for the h-reduction matmuls (M=32
    # supergroup col-tiling: group g writes scores rows 32*(g//8)+4*(g%8)+j)
    wvb = np.zeros((128, NG * 32), f32)
    for g in range(NG):
        for j in range(4):
            wvb[j * 32:(j + 1) * 32, g * 32 + 4 * (g % 8) + j] = wv

    bf16 = ml_dtypes.bfloat16
    blob_base = np.zeros((128, 336), f32)
    blob_base[0:QKD, 0:32] = Wq
    blob_base[0:QKD, 32:64] = Wk
    blob_base[QKD:128, 32:64] = Wk
    blob_base[:, 192:320] = np.eye(128, dtype=f32)
    shared = {"wvb": wvb.astype(bf16)}

    in_maps = []
    for core in range(8):
        b, half = divmod(core, 2)
        qs = queries[b, half * NQS:(half + 1) * NQS]          # (128, 64)
        # qTr[d, j*32+g] = qs[4g+j, d]
        qTr = np.ascontiguousarray(
            qs.T.reshape(QKD, NG, 4).transpose(0, 2, 1)
        ).reshape(QKD, NQS)
        kTf = keys[b].T                                        # (64, 2048)
        kT = np.ascontiguousarray(
            np.concatenate([kTf[:, 0:NK // 2], kTf[:, NK // 2:]], axis=0)
        ).astype(bf16)                                         # (128, 1024)
        v = values[b].reshape(NKT, 128, VD)
        vaug = np.concatenate([v, np.ones((NKT, 128, 1), f32)], axis=2)
        vaug = np.ascontiguousarray(vaug.transpose(1, 0, 2)).reshape(128, NKT * 65).astype(bf16)
        mask = (np.arange(NK) < int(valid_lens[b])).astype(f32)
        blob = blob_base.copy()
        blob[0:QKD, 64:192] = qTr
        blob[:, 320:336] = mask.reshape(NKT, 128).T
        in_maps.append({
            "kT": kT, "blob": blob.astype(bf16), "vaug": vaug, **shared,
        })
    return in_maps


def kernel(queries, keys, values, valid_lens, Wq, Wk, wv, _trace=False):
    if "nc" not in _cache:
        _cache["nc"] = _build_nc()
    nc = _cache["nc"]

    in_maps = _host_shards(queries, keys, values, valid_lens, Wq, Wk, wv)
    res = run_bass_kernel_spmd(nc, in_maps, core_ids=list(range(8)), trace=_trace)
    _cache["last_result"] = res

    out = np.empty((B, NQ, VD), np.float32)
    for core in range(8):
        b, half = divmod(core, 2)
        out[b, half * NQS:(half + 1) * NQS] = res.results[core]["out"]
    return out


# revision 11
# speedup vs baseline: 1.0593x; 1.0593x over previous
"""Additive (Bahdanau) attention on 8 Trainium2 NeuronCores.

Reference math (per batch b):
    qh = queries @ Wq                  (NQ, H)
    kh = keys    @ Wk                  (NK, H)
    scores[q,k] = sum_h wv[h] * tanh(qh[q,h] + kh[k,h])
    attn = softmax(mask(scores))       mask: k >= valid_len -> -1e6
    out  = attn @ values               (NQ, V)

Sharding: 8 cores = 4 batches x 2 query-halves (128 q-rows each). Each core
owns the full key dimension -> no collectives, host just concatenates.

Per-core device algorithm (NQS=128 q, NK=2048 k, H=32):
  - partitions carry (j, h) = (q mod 4, h)  -> 4*32 = 128 lanes
  - kh4 psum (128, 2048): kh replicated 4x over partition groups, computed by
    4 col-tiled matmuls lhsT=Wk rhs=keys^T
  - qh4 sbuf (128, 32): qh4[(j,h), g] = qh[4g+j, h] via 4 col-tiled matmuls
  - per q-group g (32 groups of 4 q's):
      F_g = tanh(kh4 + bias qh4[:, g])        one ScalarE pass, FD=2048
      scores[4g:4g+4, :] += wv-weighted h-reduction: TensorE matmul with a
      zero-padded (128,128) stationary weight accumulating into scores psum
  - P = exp(scores)  (no max-subtraction needed: |scores| <= ||wv||_1 ~ 5)
  - transpose P via PE, multiply by 0/1 mask column (per-partition scalar)
  - out_unnorm (128, 65) = P_T.T @ [V | 1] accumulated over 16 k-tiles;
    column 64 is the masked softmax denominator l
  - out = out_unnorm[:, :64] * (1/l)

Masked keys contribute exactly 0 (mask multiply) and the missing max
subtraction cancels in the p/l ratio, so this matches the reference exactly
up to fp32 rounding.
"""

import ml_dtypes
import numpy as np

import concourse.bacc as bacc
import concourse.tile as tile
from concourse import mybir
from concourse.bass_utils import run_bass_kernel_spmd

B, NQ, NK = 4, 256, 2048
QKD, H, VD = 64, 32, 64
NQS = 128          # q rows per core
NG = NQS // 4      # 32 q-groups of 4
NKT = NK // 128    # 16 k-tiles
F32 = mybir.dt.float32
BF16 = mybir.dt.bfloat16

_cache = {}


def _build_nc():
    nc = bacc.Bacc("TRN2", debug=False, num_devices=8)

    # blob columns: [0:32]=wq, [32:64]=wk, [64:192]=qTr, [192:320]=ident,
    # [320:336]=maskc  (wq/wk/qTr occupy partitions 0-63)
    d_kT = nc.declare_dram_parameter("kT", [128, NK // 2], BF16, isOutput=False)
    d_blob = nc.declare_dram_parameter("blob", [128, 336], BF16, isOutput=False)
    d_wvb = nc.declare_dram_parameter("wvb", [128, NG * 32], BF16, isOutput=False)
    d_vaug = nc.declare_dram_parameter("vaug", [128, NKT * 65], BF16, isOutput=False)
    d_out = nc.declare_dram_parameter("out", [NQS, VD], F32, isOutput=True)

    TANH = mybir.ActivationFunctionType.Tanh
    EXP = mybir.ActivationFunctionType.Exp

    with tile.TileContext(nc) as tc:
        with (
            tc.tile_pool(name="sb", bufs=1) as sb,
            tc.tile_pool(name="fpool", bufs=2) as fpool,
            tc.tile_pool(name="psA", bufs=1, space="PSUM") as psA,
            tc.tile_pool(name="psB", bufs=1, space="PSUM") as psB,
        ):
            # ---- constant / input tiles ----
            # kT2: [0:64, f] = keys^T[:, f], [64:128, f] = keys^T[:, 1024+f]
            kT_sb = sb.tile([128, NK // 2], BF16, tag="kT")
            blob_sb = sb.tile([128, 336], BF16, tag="blob")
            wvb_sb = sb.tile([128, NG * 32], BF16, tag="wvb")
            vaug_sb = sb.tile([128, NKT * 65], BF16, tag="vaug")
            qh4_sb = sb.tile([128, NG], F32, tag="qh4")
            kh4bf_sb = sb.tile([128, NK], BF16, tag="kh4bf")
            wq_sb = blob_sb[0:QKD, 0:32]
            wk_lo = blob_sb[0:QKD, 32:64]
            wk_hi = blob_sb[QKD:128, 32:64]
            qTr_sb = blob_sb[0:QKD, 64:192]
            ident_sb = blob_sb[:, 192:320]
            maskc_bf = blob_sb[:, 320:336]
            maskc_sb = sb.tile([128, NKT], F32, tag="maskf")
            P_sb = sb.tile([128, NK], BF16, tag="P")
            PT_sb = sb.tile([128, NK], BF16, tag="PT")
            linv_sb = sb.tile([128, 1], F32, tag="linv")
            out_sb = sb.tile([NQS, VD], F32, tag="outsb")

            for c in range(2):
                nc.sync.dma_start(
                    out=kT_sb[:, c * 512:(c + 1) * 512],
                    in_=d_kT[:, c * 512:(c + 1) * 512],
                )
            nc.gpsimd.dma_start(out=blob_sb[:], in_=d_blob[:])
            nc.gpsimd.dma_start(out=wvb_sb[:], in_=d_wvb[:])
            nc.gpsimd.dma_start(out=vaug_sb[:], in_=d_vaug[:])
            nc.vector.tensor_copy(maskc_sb[:], maskc_bf)

            # ---- psum tiles ----
            kh4_ps = psA.tile([128, NK], F32, tag="big")
            qh4_ps = psB.tile([128, NG], F32, tag="acc")

            # qh4[(j,h), g] = sum_d Wq[d,h] * qTr[d, j*32+g]
            for j in range(4):
                nc.tensor.matmul(
                    qh4_ps[32 * j:32 * (j + 1), :],
                    lhsT=wq_sb,
                    rhs=qTr_sb[:, j * 32:(j + 1) * 32],
                    start=True, stop=True,
                    tile_position=(0, 32 * j),
                )
            nc.vector.tensor_copy(qh4_sb[:], qh4_ps[:])

            # kh4[(j,h), k] = sum_d Wk[d,h] * kT[d,k]  (replicated over j),
            # then narrowed to bf16 in SBUF per 512-chunk
            for c in range(4):
                src_rows = 0 if c < 2 else QKD
                rhs = kT_sb[src_rows:src_rows + QKD, (c % 2) * 512:(c % 2 + 1) * 512]
                wk = wk_lo if c < 2 else wk_hi
                for j in range(4):
                    nc.tensor.matmul(
                        kh4_ps[32 * j:32 * (j + 1), c * 512:(c + 1) * 512],
                        lhsT=wk,
                        rhs=rhs,
                        start=True, stop=True,
                        tile_position=(src_rows, 32 * j),
                    )
                nc.vector.tensor_copy(
                    kh4bf_sb[:, c * 512:(c + 1) * 512],
                    kh4_ps[:, c * 512:(c + 1) * 512],
                )

            # ---- main loop: DVE bias-add -> one big in-place tanh per chunk
            # -> TensorE h-reduction. Ramped chunk sizes keep startup short.
            scores_ps = psB.tile([128, NK], F32, tag="acc")
            CHUNKS = [2] * 16
            g = 0
            for nch in CHUNKS:
                Fs = fpool.tile([128, nch * NK], BF16, tag=f"Fs{nch}",
                                bufs=4, name=f"Fs_{g}")
                for i in range(nch):
                    nc.vector.tensor_scalar_add(
                        Fs[:, i * NK:(i + 1) * NK], kh4bf_sb[:],
                        qh4_sb[:, g + i:g + i + 1],
                    )
                nc.scalar.activation(Fs[:], Fs[:], TANH)
                for i in range(nch):
                    gg = g + i
                    G = gg // 8
                    for c in range(4):
                        nc.tensor.matmul(
                            scores_ps[32 * G:32 * (G + 1), c * 512:(c + 1) * 512],
                            lhsT=wvb_sb[:, gg * 32:(gg + 1) * 32],
                            rhs=Fs[:, i * NK + c * 512:i * NK + (c + 1) * 512],
                            start=(gg % 8 == 0), stop=(gg % 8 == 7),
                            skip_group_check=True,
                            tile_position=(0, 32 * G),
                        )
                g += nch

            # ---- softmax numerator ----
            nc.scalar.activation(P_sb[:], scores_ps[:], EXP)

            # ---- transpose P (PE) + mask multiply (DVE) + AV matmul ----
            PT_ps = psA.tile([128, 2 * NK], BF16, tag="big")
            av_ps = psB.tile([128, 65], F32, tag="acc")
            for t in range(NKT):
                off = (t % 4) * 1024 + (t // 4) * 128
                nc.tensor.transpose(
                    PT_ps[:, off:off + 128],
                    P_sb[:, t * 128:(t + 1) * 128],
                    ident_sb,
                )
                nc.vector.tensor_scalar_mul(
                    PT_sb[:, t * 128:(t + 1) * 128],
                    PT_ps[:, off:off + 128],
                    maskc_sb[:, t:t + 1],
                )
                nc.tensor.matmul(
                    av_ps[:],
                    lhsT=PT_sb[:, t * 128:(t + 1) * 128],
                    rhs=vaug_sb[:, t * 65:(t + 1) * 65],
                    start=(t == 0), stop=(t == NKT - 1),
                )

            # ---- normalize + store ----
            nc.vector.reciprocal(linv_sb[:], av_ps[:, 64:65])
            nc.vector.tensor_scalar_mul(out_sb[:], av_ps[:, 0:64], linv_sb[:])
            nc.sync.dma_start(out=d_out[:], in_=out_sb[:])

    nc.compile()
    return nc


def _host_shards(queries, keys, values, valid_lens, Wq, Wk, wv):
    """Pure data-marshaling: shard, transpose layouts, build mask/weight
    layouts. All FLOPs on the actual tensors happen on device."""
    f32 = np.float32
    queries = np.asarray(queries, f32)
    keys = np.asarray(keys, f32)
    values = np.asarray(values, f32)
    valid_lens = np.asarray(valid_lens)
    Wq = np.asarray(Wq, f32)
    Wk = np.asarray(Wk, f32)
    wv = np.asarray(wv, f32)

    # zero-padded stationary weights for the h-reduction matmuls (M=32
    # supergroup col-tiling: group g writes scores rows 32*(g//8)+4*(g%8)+j)
    wvb = np.zeros((128, NG * 32), f32)
    for g in range(NG):
        for j in range(4):
            wvb[j * 32:(j + 1) * 32, g * 32 + 4 * (g % 8) + j] = wv

    bf16 = ml_dtypes.bfloat16
    blob_base = np.zeros((128, 336), f32)
    blob_base[0:QKD, 0:32] = Wq
    blob_base[0:QKD, 32:64] = Wk
    blob_base[QKD:128, 32:64] = Wk
    blob_base[:, 192:320] = np.eye(128, dtype=f32)
    shared = {"wvb": wvb.astype(bf16)}

    in_maps = []
    for core in range(8):
        b, half = divmod(core, 2)
        qs = queries[b, half * NQS:(half + 1) * NQS]          # (128, 64)
        # qTr[d, j*32+g] = qs[4g+j, d]
        qTr = np.ascontiguousarray(
            qs.T.reshape(QKD, NG, 4).transpose(0, 2, 1)
        ).reshape(QKD, NQS)
        kTf = keys[b].T                                        # (64, 2048)
        kT = np.ascontiguousarray(
            np.concatenate([kTf[:, 0:NK // 2], kTf[:, NK // 2:]], axis=0)
        ).astype(bf16)                                         # (128, 1024)
        v = values[b].reshape(NKT, 128, VD)
        vaug = np.concatenate([v, np.ones((NKT, 128, 1), f32)], axis=2)
        vaug = np.ascontiguousarray(vaug.transpose(1, 0, 2)).reshape(128, NKT * 65).astype(bf16)
        mask = (np.arange(NK) < int(valid_lens[b])).astype(f32)
        blob = blob_base.copy()
        blob[0:QKD, 64:192] = qTr
        blob[:, 320:336] = mask.reshape(NKT, 128).T
        in_maps.append({
            "kT": kT, "blob": blob.astype(bf16), "vaug": vaug, **shared,
        })
    return in_maps


def kernel(queries, keys, values, valid_lens, Wq, Wk, wv, _trace=False):
    if "nc" not in _cache:
        _cache["nc"] = _build_nc()
    nc = _cache["nc"]

    in_maps = _host_shards(queries, keys, values, valid_lens, Wq, Wk, wv)
    res = run_bass_kernel_spmd(nc, in_maps, core_ids=list(range(8)), trace=_trace)
    _cache["last_result"] = res

    out = np.empty((B, NQ, VD), np.float32)
    for core in range(8):
        b, half = divmod(core, 2)
        out[b, half * NQS:(half + 1) * NQS] = res.results[core]["out"]
    return out


# revision 13
# speedup vs baseline: 1.1809x; 1.1148x over previous
"""Additive (Bahdanau) attention on 8 Trainium2 NeuronCores.

Reference math (per batch b):
    qh = queries @ Wq                  (NQ, H)
    kh = keys    @ Wk                  (NK, H)
    scores[q,k] = sum_h wv[h] * tanh(qh[q,h] + kh[k,h])
    attn = softmax(mask(scores))       mask: k >= valid_len -> -1e6
    out  = attn @ values               (NQ, V)

Sharding: 8 cores = 4 batches x 2 query-halves (128 q-rows each). Each core
owns the full key dimension -> no collectives, host just concatenates.

Per-core device algorithm (NQS=128 q, NK=2048 k, H=32):
  - partitions carry (j, h) = (q mod 4, h)  -> 4*32 = 128 lanes
  - kh4 psum (128, 2048): kh replicated 4x over partition groups, computed by
    4 col-tiled matmuls lhsT=Wk rhs=keys^T
  - qh4 sbuf (128, 32): qh4[(j,h), g] = qh[4g+j, h] via 4 col-tiled matmuls
  - per q-group g (32 groups of 4 q's):
      F_g = tanh(kh4 + bias qh4[:, g])        one ScalarE pass, FD=2048
      scores[4g:4g+4, :] += wv-weighted h-reduction: TensorE matmul with a
      zero-padded (128,128) stationary weight accumulating into scores psum
  - P = exp(scores)  (no max-subtraction needed: |scores| <= ||wv||_1 ~ 5)
  - transpose P via PE, multiply by 0/1 mask column (per-partition scalar)
  - out_unnorm (128, 65) = P_T.T @ [V | 1] accumulated over 16 k-tiles;
    column 64 is the masked softmax denominator l
  - out = out_unnorm[:, :64] * (1/l)

Masked keys contribute exactly 0 (mask multiply) and the missing max
subtraction cancels in the p/l ratio, so this matches the reference exactly
up to fp32 rounding.
"""

import ml_dtypes
import numpy as np

import concourse.bacc as bacc
import concourse.tile as tile
from concourse import mybir
from concourse.bass_utils import run_bass_kernel_spmd

B, NQ, NK = 4, 256, 2048
QKD, H, VD = 64, 32, 64
NQS = 128          # q rows per core
NG = NQS // 4      # 32 q-groups of 4
NKT = NK // 128    # 16 k-tiles
F32 = mybir.dt.float32
BF16 = mybir.dt.bfloat16

_cache = {}


def _build_nc():
    nc = bacc.Bacc("TRN2", debug=False, num_devices=8)

    # blob columns: [0:32]=wq, [32:64]=wk, [64:192]=qTr, [192:320]=ident,
    # [320:336]=maskc  (wq/wk/qTr occupy partitions 0-63)
    d_kT = nc.declare_dram_parameter("kT", [128, NK // 2], BF16, isOutput=False)
    d_blob = nc.declare_dram_parameter("blob", [128, 336], BF16, isOutput=False)
    d_wvb = nc.declare_dram_parameter("wvb", [128, NG * 32], BF16, isOutput=False)
    d_vaug = nc.declare_dram_parameter("vaug", [128, NKT * 65], BF16, isOutput=False)
    d_out = nc.declare_dram_parameter("out", [NQS, VD], F32, isOutput=True)

    TANH = mybir.ActivationFunctionType.Tanh
    EXP = mybir.ActivationFunctionType.Exp

    with tile.TileContext(nc) as tc:
        with (
            tc.tile_pool(name="sb", bufs=1) as sb,
            tc.tile_pool(name="fpool", bufs=2) as fpool,
            tc.tile_pool(name="psA", bufs=1, space="PSUM") as psA,
            tc.tile_pool(name="psB", bufs=1, space="PSUM") as psB,
        ):
            # ---- constant / input tiles ----
            # kT2: [0:64, f] = keys^T[:, f], [64:128, f] = keys^T[:, 1024+f]
            kT_sb = sb.tile([128, NK // 2], BF16, tag="kT")
            blob_sb = sb.tile([128, 336], BF16, tag="blob")
            wvb_sb = sb.tile([128, NG * 32], BF16, tag="wvb")
            vaug_sb = sb.tile([128, NKT * 65], BF16, tag="vaug")
            qh4_sb = sb.tile([128, NG], F32, tag="qh4")
            kh4bf_sb = sb.tile([128, NK], BF16, tag="kh4bf")
            wq_sb = blob_sb[0:QKD, 0:32]
            wk_lo = blob_sb[0:QKD, 32:64]
            wk_hi = blob_sb[QKD:128, 32:64]
            qTr_sb = blob_sb[0:QKD, 64:192]
            ident_sb = blob_sb[:, 192:320]
            maskc_bf = blob_sb[:, 320:336]
            maskc_sb = sb.tile([128, NKT], F32, tag="maskf")
            P_sb = sb.tile([128, NK], BF16, tag="P")
            PT_sb = sb.tile([128, NK], BF16, tag="PT")
            linv_sb = sb.tile([128, 1], F32, tag="linv")
            out_sb = sb.tile([NQS, VD], F32, tag="outsb")

            nc.scalar.dma_start(out=kT_sb[:], in_=d_kT[:])
            nc.scalar.dma_start(out=blob_sb[:], in_=d_blob[:])
            nc.gpsimd.dma_start(out=wvb_sb[:], in_=d_wvb[:])
            nc.gpsimd.dma_start(out=vaug_sb[:], in_=d_vaug[:])
            nc.vector.tensor_copy(maskc_sb[:], maskc_bf)

            # ---- psum tiles ----
            kh4c = [psA.tile([128, 512], F32, tag=f"big{c}", name=f"kh4c{c}")
                    for c in range(4)]
            qh4_ps = psB.tile([128, NG], F32, tag="acc")

            # qh4[(j,h), g] = sum_d Wq[d,h] * qTr[d, j*32+g]
            for j in range(4):
                nc.tensor.matmul(
                    qh4_ps[32 * j:32 * (j + 1), :],
                    lhsT=wq_sb,
                    rhs=qTr_sb[:, j * 32:(j + 1) * 32],
                    start=True, stop=True,
                    tile_position=(0, 32 * j),
                )
            nc.vector.tensor_copy(qh4_sb[:], qh4_ps[:])

            # kh4[(j,h), k] = sum_d Wk[d,h] * kT[d,k]  (replicated over j),
            # then narrowed to bf16 in SBUF per 512-chunk
            for c in range(4):
                src_rows = 0 if c < 2 else QKD
                rhs = kT_sb[src_rows:src_rows + QKD, (c % 2) * 512:(c % 2 + 1) * 512]
                wk = wk_lo if c < 2 else wk_hi
                for j in range(4):
                    nc.tensor.matmul(
                        kh4c[c][32 * j:32 * (j + 1), :],
                        lhsT=wk,
                        rhs=rhs,
                        start=True, stop=True,
                        tile_position=(src_rows, 32 * j),
                    )
                nc.vector.tensor_copy(
                    kh4bf_sb[:, c * 512:(c + 1) * 512],
                    kh4c[c][:],
                )

            # ---- main loop: DVE bias-add -> one big in-place tanh per chunk
            # -> TensorE h-reduction. Ramped chunk sizes keep startup short.
            scores_ps = psB.tile([128, NK], F32, tag="acc")
            CHUNKS = [2] * 15 + [1, 1]
            g = 0
            for nch in CHUNKS:
                Fs = fpool.tile([128, nch * NK], BF16, tag=f"Fs{nch}",
                                bufs=4, name=f"Fs_{g}")
                for i in range(nch):
                    nc.vector.tensor_scalar_add(
                        Fs[:, i * NK:(i + 1) * NK], kh4bf_sb[:],
                        qh4_sb[:, g + i:g + i + 1],
                    )
                nc.scalar.activation(Fs[:], Fs[:], TANH)
                for i in range(nch):
                    gg = g + i
                    G = gg // 8
                    for c in range(4):
                        nc.tensor.matmul(
                            scores_ps[32 * G:32 * (G + 1), c * 512:(c + 1) * 512],
                            lhsT=wvb_sb[:, gg * 32:(gg + 1) * 32],
                            rhs=Fs[:, i * NK + c * 512:i * NK + (c + 1) * 512],
                            start=(gg % 8 == 0), stop=(gg % 8 == 7),
                            skip_group_check=True,
                            tile_position=(0, 32 * G),
                        )
                g += nch

            # ---- softmax numerator ----
            nc.scalar.activation(P_sb[:], scores_ps[:], EXP)

            # ---- transpose P (PE) + mask multiply (DVE) + AV matmul ----
            PTb = [psA.tile([128, 1024], BF16, tag=f"big{i}", name=f"PTb{i}")
                   for i in range(4)]
            av_ps = psB.tile([128, 65], F32, tag="acc")
            for t in range(NKT):
                pt = PTb[t % 4][:, (t // 4) * 128:(t // 4 + 1) * 128]
                nc.tensor.transpose(
                    pt,
                    P_sb[:, t * 128:(t + 1) * 128],
                    ident_sb,
                )
                nc.vector.tensor_scalar_mul(
                    PT_sb[:, t * 128:(t + 1) * 128],
                    pt,
                    maskc_sb[:, t:t + 1],
                )
                nc.tensor.matmul(
                    av_ps[:],
                    lhsT=PT_sb[:, t * 128:(t + 1) * 128],
                    rhs=vaug_sb[:, t * 65:(t + 1) * 65],
                    start=(t == 0), stop=(t == NKT - 1),
                )

            # ---- normalize + store ----
            nc.vector.reciprocal(linv_sb[:], av_ps[:, 64:65])
            nc.vector.tensor_scalar_mul(out_sb[:], av_ps[:, 0:64], linv_sb[:])
            nc.sync.dma_start(out=d_out[:], in_=out_sb[:])

    nc.compile()
    return nc


def _host_shards(queries, keys, values, valid_lens, Wq, Wk, wv):
    """Pure data-marshaling: shard, transpose layouts, build mask/weight
    layouts. All FLOPs on the actual tensors happen on device."""
    f32 = np.float32
    queries = np.asarray(queries, f32)
    keys = np.asarray(keys, f32)
    values = np.asarray(values, f32)
    valid_lens = np.asarray(valid_lens)
    Wq = np.asarray(Wq, f32)
    Wk = np.asarray(Wk, f32)
    wv = np.asarray(wv, f32)

    # zero-padded stationary weights for the h-reduction matmuls (M=32
    # supergroup col-tiling: group g writes scores rows 32*(g//8)+4*(g%8)+j)
    wvb = np.zeros((128, NG * 32), f32)
    for g in range(NG):
        for j in range(4):
            wvb[j * 32:(j + 1) * 32, g * 32 + 4 * (g % 8) + j] = wv

    bf16 = ml_dtypes.bfloat16
    blob_base = np.zeros((128, 336), f32)
    blob_base[0:QKD, 0:32] = Wq
    blob_base[0:QKD, 32:64] = Wk
    blob_base[QKD:128, 32:64] = Wk
    blob_base[:, 192:320] = np.eye(128, dtype=f32)
    shared = {"wvb": wvb.astype(bf16)}

    in_maps = []
    for core in range(8):
        b, half = divmod(core, 2)
        qs = queries[b, half * NQS:(half + 1) * NQS]          # (128, 64)
        # qTr[d, j*32+g] = qs[4g+j, d]
        qTr = np.ascontiguousarray(
            qs.T.reshape(QKD, NG, 4).transpose(0, 2, 1)
        ).reshape(QKD, NQS)
        kTf = keys[b].T                                        # (64, 2048)
        kT = np.ascontiguousarray(
            np.concatenate([kTf[:, 0:NK // 2], kTf[:, NK // 2:]], axis=0)
        ).astype(bf16)                                         # (128, 1024)
        v = values[b].reshape(NKT, 128, VD)
        vaug = np.concatenate([v, np.ones((NKT, 128, 1), f32)], axis=2)
        vaug = np.ascontiguousarray(vaug.transpose(1, 0, 2)).reshape(128, NKT * 65).astype(bf16)
        mask = (np.arange(NK) < int(valid_lens[b])).astype(f32)
        blob = blob_base.copy()
        blob[0:QKD, 64:192] = qTr
        blob[:, 320:336] = mask.reshape(NKT, 128).T
        in_maps.append({
            "kT": kT, "blob": blob.astype(bf16), "vaug": vaug, **shared,
        })
    return in_maps


def kernel(queries, keys, values, valid_lens, Wq, Wk, wv, _trace=False):
    if "nc" not in _cache:
        _cache["nc"] = _build_nc()
    nc = _cache["nc"]

    in_maps = _host_shards(queries, keys, values, valid_lens, Wq, Wk, wv)
    res = run_bass_kernel_spmd(nc, in_maps, core_ids=list(range(8)), trace=_trace)
    _cache["last_result"] = res

    out = np.empty((B, NQ, VD), np.float32)
    for core in range(8):
        b, half = divmod(core, 2)
        out[b, half * NQS:(half + 1) * NQS] = res.results[core]["out"]
    return out


# revision 14
# speedup vs baseline: 1.2268x; 1.0389x over previous
"""Additive (Bahdanau) attention on 8 Trainium2 NeuronCores.

Reference math (per batch b):
    qh = queries @ Wq                  (NQ, H)
    kh = keys    @ Wk                  (NK, H)
    scores[q,k] = sum_h wv[h] * tanh(qh[q,h] + kh[k,h])
    attn = softmax(mask(scores))       mask: k >= valid_len -> -1e6
    out  = attn @ values               (NQ, V)

Sharding: 8 cores = 4 batches x 2 query-halves (128 q-rows each). Each core
owns the full key dimension -> no collectives, host just concatenates.

Per-core device algorithm (NQS=128 q, NK=2048 k, H=32):
  - partitions carry (j, h) = (q mod 4, h)  -> 4*32 = 128 lanes
  - kh4 psum (128, 2048): kh replicated 4x over partition groups, computed by
    4 col-tiled matmuls lhsT=Wk rhs=keys^T
  - qh4 sbuf (128, 32): qh4[(j,h), g] = qh[4g+j, h] via 4 col-tiled matmuls
  - per q-group g (32 groups of 4 q's):
      F_g = tanh(kh4 + bias qh4[:, g])        one ScalarE pass, FD=2048
      scores[4g:4g+4, :] += wv-weighted h-reduction: TensorE matmul with a
      zero-padded (128,128) stationary weight accumulating into scores psum
  - P = exp(scores)  (no max-subtraction needed: |scores| <= ||wv||_1 ~ 5)
  - transpose P via PE, multiply by 0/1 mask column (per-partition scalar)
  - out_unnorm (128, 65) = P_T.T @ [V | 1] accumulated over 16 k-tiles;
    column 64 is the masked softmax denominator l
  - out = out_unnorm[:, :64] * (1/l)

Masked keys contribute exactly 0 (mask multiply) and the missing max
subtraction cancels in the p/l ratio, so this matches the reference exactly
up to fp32 rounding.
"""

import ml_dtypes
import numpy as np

import concourse.bacc as bacc
import concourse.tile as tile
from concourse import mybir
from concourse.bass_utils import run_bass_kernel_spmd

B, NQ, NK = 4, 256, 2048
QKD, H, VD = 64, 32, 64
NQS = 128          # q rows per core
NG = NQS // 4      # 32 q-groups of 4
NKT = NK // 128    # 16 k-tiles
F32 = mybir.dt.float32
BF16 = mybir.dt.bfloat16

_cache = {}


def _build_nc():
    nc = bacc.Bacc("TRN2", debug=False, num_devices=8)

    # blob columns: [0:32]=wq, [32:64]=wk, [64:192]=qTr, [192:320]=ident,
    # [320:336]=maskc  (wq/wk/qTr occupy partitions 0-63)
    d_kT = nc.declare_dram_parameter("kT", [128, NK // 2], BF16, isOutput=False)
    d_blob = nc.declare_dram_parameter("blob", [128, 336], BF16, isOutput=False)
    d_wvb = nc.declare_dram_parameter("wvb", [128, NG * 32], BF16, isOutput=False)
    d_vaug = nc.declare_dram_parameter("vaug", [128, NKT * 65], BF16, isOutput=False)
    d_out = nc.declare_dram_parameter("out", [NQS, VD], F32, isOutput=True)

    TANH = mybir.ActivationFunctionType.Tanh
    EXP = mybir.ActivationFunctionType.Exp

    with tile.TileContext(nc) as tc:
        with (
            tc.tile_pool(name="sb", bufs=1) as sb,
            tc.tile_pool(name="fpool", bufs=2) as fpool,
            tc.tile_pool(name="psA", bufs=1, space="PSUM") as psA,
            tc.tile_pool(name="psB", bufs=1, space="PSUM") as psB,
        ):
            # ---- constant / input tiles ----
            # kT2: [0:64, f] = keys^T[:, f], [64:128, f] = keys^T[:, 1024+f]
            kT_sb = sb.tile([128, NK // 2], BF16, tag="kT")
            blob_sb = sb.tile([128, 336], BF16, tag="blob")
            wvb_sb = sb.tile([128, NG * 32], BF16, tag="wvb")
            vaug_sb = sb.tile([128, NKT * 65], BF16, tag="vaug")
            qh4_sb = sb.tile([128, NG], F32, tag="qh4")
            kh4bf_sb = sb.tile([128, NK], BF16, tag="kh4bf")
            wq_sb = blob_sb[0:QKD, 0:32]
            wk_lo = blob_sb[0:QKD, 32:64]
            wk_hi = blob_sb[QKD:128, 32:64]
            qTr_sb = blob_sb[0:QKD, 64:192]
            ident_sb = blob_sb[:, 192:320]
            maskc_bf = blob_sb[:, 320:336]
            maskc_sb = sb.tile([128, NKT], F32, tag="maskf")
            P_sb = sb.tile([128, NK], BF16, tag="P")
            PT_sb = sb.tile([128, NK], BF16, tag="PT")
            linv_sb = sb.tile([128, 1], F32, tag="linv")
            out_sb = sb.tile([NQS, VD], F32, tag="outsb")

            for c in range(2):
                nc.sync.dma_start(
                    out=kT_sb[:, c * 512:(c + 1) * 512],
                    in_=d_kT[:, c * 512:(c + 1) * 512],
                )
            nc.scalar.dma_start(out=blob_sb[:], in_=d_blob[:])
            nc.vector.tensor_copy(maskc_sb[:], maskc_bf)

            # ---- psum tiles ----
            kh4c = [psA.tile([128, 512], F32, tag=f"big{c}", name=f"kh4c{c}")
                    for c in range(4)]
            qh4_ps = psB.tile([128, NG], F32, tag="acc")

            # qh4[(j,h), g] = sum_d Wq[d,h] * qTr[d, j*32+g]
            for j in range(4):
                nc.tensor.matmul(
                    qh4_ps[32 * j:32 * (j + 1), :],
                    lhsT=wq_sb,
                    rhs=qTr_sb[:, j * 32:(j + 1) * 32],
                    start=True, stop=True,
                    tile_position=(0, 32 * j),
                )
            nc.vector.tensor_copy(qh4_sb[:], qh4_ps[:])

            # kh4[(j,h), k] = sum_d Wk[d,h] * kT[d,k]  (replicated over j),
            # then narrowed to bf16 in SBUF per 512-chunk
            for c in (0, 2, 1, 3):
                src_rows = 0 if c < 2 else QKD
                rhs = kT_sb[src_rows:src_rows + QKD, (c % 2) * 512:(c % 2 + 1) * 512]
                wk = wk_lo if c < 2 else wk_hi
                for j in range(4):
                    nc.tensor.matmul(
                        kh4c[c][32 * j:32 * (j + 1), :],
                        lhsT=wk,
                        rhs=rhs,
                        start=True, stop=True,
                        tile_position=(src_rows, 32 * j),
                    )
                nc.vector.tensor_copy(
                    kh4bf_sb[:, c * 512:(c + 1) * 512],
                    kh4c[c][:],
                )

            # ---- main loop: DVE bias-add -> one big in-place tanh per chunk
            # -> TensorE h-reduction. Ramped chunk sizes keep startup short.
            nc.gpsimd.dma_start(out=wvb_sb[:], in_=d_wvb[:])

            scores_ps = psB.tile([128, NK], F32, tag="acc")
            CHUNKS = [2, 2, 4, 4, 4, 4, 4, 4, 2, 1, 1]
            g = 0
            for nch in CHUNKS:
                Fs = fpool.tile([128, nch * NK], BF16, tag=f"Fs{nch}",
                                bufs=4, name=f"Fs_{g}")
                for half in range(2):
                    for i in range(nch):
                        nc.vector.tensor_scalar_add(
                            Fs[:, i * NK + half * 1024:i * NK + (half + 1) * 1024],
                            kh4bf_sb[:, half * 1024:(half + 1) * 1024],
                            qh4_sb[:, g + i:g + i + 1],
                        )
                nc.scalar.activation(Fs[:], Fs[:], TANH)
                for i in range(nch):
                    gg = g + i
                    G = gg // 8
                    for c in range(4):
                        nc.tensor.matmul(
                            scores_ps[32 * G:32 * (G + 1), c * 512:(c + 1) * 512],
                            lhsT=wvb_sb[:, gg * 32:(gg + 1) * 32],
                            rhs=Fs[:, i * NK + c * 512:i * NK + (c + 1) * 512],
                            start=(gg % 8 == 0), stop=(gg % 8 == 7),
                            skip_group_check=True,
                            tile_position=(0, 32 * G),
                        )
                g += nch

            nc.gpsimd.dma_start(out=vaug_sb[:], in_=d_vaug[:])

            # ---- softmax numerator ----
            nc.scalar.activation(P_sb[:, 0:1024], scores_ps[:, 0:1024], EXP)
            nc.scalar.activation(P_sb[:, 1024:2048], scores_ps[:, 1024:2048], EXP)

            # ---- transpose P (PE) + mask multiply (DVE) + AV matmul ----
            PTb = [psA.tile([128, 1024], BF16, tag=f"big{i}", name=f"PTb{i}")
                   for i in range(4)]
            av_ps = psB.tile([128, 65], F32, tag="acc")
            for t in range(NKT):
                pt = PTb[t % 4][:, (t // 4) * 128:(t // 4 + 1) * 128]
                nc.tensor.transpose(
                    pt,
                    P_sb[:, t * 128:(t + 1) * 128],
                    ident_sb,
                )
                nc.vector.tensor_scalar_mul(
                    PT_sb[:, t * 128:(t + 1) * 128],
                    pt,
                    maskc_sb[:, t:t + 1],
                )
                nc.tensor.matmul(
                    av_ps[:],
                    lhsT=PT_sb[:, t * 128:(t + 1) * 128],
                    rhs=vaug_sb[:, t * 65:(t + 1) * 65],
                    start=(t == 0), stop=(t == NKT - 1),
                )

            # ---- normalize + store ----
            nc.vector.reciprocal(linv_sb[:], av_ps[:, 64:65])
            nc.vector.tensor_scalar_mul(out_sb[:], av_ps[:, 0:64], linv_sb[:])
            nc.sync.dma_start(out=d_out[:], in_=out_sb[:])

    nc.compile()
    return nc


def _host_shards(queries, keys, values, valid_lens, Wq, Wk, wv):
    """Pure data-marshaling: shard, transpose layouts, build mask/weight
    layouts. All FLOPs on the actual tensors happen on device."""
    f32 = np.float32
    queries = np.asarray(queries, f32)
    keys = np.asarray(keys, f32)
    values = np.asarray(values, f32)
    valid_lens = np.asarray(valid_lens)
    Wq = np.asarray(Wq, f32)
    Wk = np.asarray(Wk, f32)
    wv = np.asarray(wv, f32)

    # zero-padded stationary weights for the h-reduction matmuls (M=32
    # supergroup col-tiling: group g writes scores rows 32*(g//8)+4*(g%8)+j)
    wvb = np.zeros((128, NG * 32), f32)
    for g in range(NG):
        for j in range(4):
            wvb[j * 32:(j + 1) * 32, g * 32 + 4 * (g % 8) + j] = wv

    bf16 = ml_dtypes.bfloat16
    blob_base = np.zeros((128, 336), f32)
    blob_base[0:QKD, 0:32] = Wq
    blob_base[0:QKD, 32:64] = Wk
    blob_base[QKD:128, 32:64] = Wk
    blob_base[:, 192:320] = np.eye(128, dtype=f32)
    shared = {"wvb": wvb.astype(bf16)}

    in_maps = []
    for core in range(8):
        b, half = divmod(core, 2)
        qs = queries[b, half * NQS:(half + 1) * NQS]          # (128, 64)
        # qTr[d, j*32+g] = qs[4g+j, d]
        qTr = np.ascontiguousarray(
            qs.T.reshape(QKD, NG, 4).transpose(0, 2, 1)
        ).reshape(QKD, NQS)
        kTf = keys[b].T                                        # (64, 2048)
        kT = np.ascontiguousarray(
            np.concatenate([kTf[:, 0:NK // 2], kTf[:, NK // 2:]], axis=0)
        ).astype(bf16)                                         # (128, 1024)
        v = values[b].reshape(NKT, 128, VD)
        vaug = np.concatenate([v, np.ones((NKT, 128, 1), f32)], axis=2)
        vaug = np.ascontiguousarray(vaug.transpose(1, 0, 2)).reshape(128, NKT * 65).astype(bf16)
        mask = (np.arange(NK) < int(valid_lens[b])).astype(f32)
        blob = blob_base.copy()
        blob[0:QKD, 64:192] = qTr
        blob[:, 320:336] = mask.reshape(NKT, 128).T
        in_maps.append({
            "kT": kT, "blob": blob.astype(bf16), "vaug": vaug, **shared,
        })
    return in_maps


def kernel(queries, keys, values, valid_lens, Wq, Wk, wv, _trace=False):
    if "nc" not in _cache:
        _cache["nc"] = _build_nc()
    nc = _cache["nc"]

    in_maps = _host_shards(queries, keys, values, valid_lens, Wq, Wk, wv)
    res = run_bass_kernel_spmd(nc, in_maps, core_ids=list(range(8)), trace=_trace)
    _cache["last_result"] = res

    out = np.empty((B, NQ, VD), np.float32)
    for core in range(8):
        b, half = divmod(core, 2)
        out[b, half * NQS:(half + 1) * NQS] = res.results[core]["out"]
    return out
